# revision 11
# baseline (speedup 1.0000x reference)
"""GATv2Conv forward on 8 Trainium2 NeuronCores (Bass/Tile).

Strategy (dst-sharded, edge-gather, no collectives):
  - Host sorts edges by destination node; core k owns dst nodes
    [k*12544, (k+1)*12544).  Each core processes its own edges fully
    independently (segment max is skipped: scores are small enough that
    exp() cannot overflow, and softmax is shift-invariant).
  - Per dst tile of 128 nodes, edges are packed into chunks of 128
    (padded; pad edges carry dst_local=200 so they match no one-hot
    column and contribute nothing).
  - Per edge chunk:
      gather x[src], x[dst] rows (indirect DMA)  -> [128e, 128]
      PE transpose                               -> xT [128feat, 128e]
      sT = W_comb^T-ish matmuls                  -> [ch, e] blocks (PSUM)
      e_scores = 0.8*att.relu(s) + 0.2*att.s  via PE matmuls into [8, e]
      exp on ACT, PE transpose back              -> ex [128e, 8]
      denom += onehot^T @ ex ; agg += onehot^T @ (ex (x) x_src)
  - Per tile: normalize agg by 1/denom, transpose, multiply by
    block-diagonal W_l, add bias, DMA out.
Output is assembled (concat + crop) on the host.
"""

import sys

sys.path.insert(0, "/opt/trn_rl_repo")

import numpy as np

N_NODES = 100000
N_EDGES = 800000
IN_DIM = 64
HEADS = 8
OUT_C = 64
HC = HEADS * OUT_C  # 512
NEG = 0.2
P = 128
NCORES = 8
TILES = 98  # dst tiles per core
NPC = TILES * P  # 12544 nodes per core
NPAD = NPC * NCORES  # 100352
CHUNK_CAP = 40  # sanity bound on chunks per tile

_COMPILED = {}  # (TC, chunks tuple) -> (nc, runner)


# ----------------------------------------------------------------- host prep
def _preprocess(x, edge_index, W_l, W_r, att, bias):
    x = np.ascontiguousarray(np.asarray(x, dtype=np.float32))
    ei = np.asarray(edge_index)
    src = np.ascontiguousarray(ei[0]).astype(np.int64)
    dst = np.ascontiguousarray(ei[1]).astype(np.int64)
    W_l = np.asarray(W_l, dtype=np.float32)
    W_r = np.asarray(W_r, dtype=np.float32)
    att = np.asarray(att, dtype=np.float32)
    bias = np.asarray(bias, dtype=np.float32)

    E = src.shape[0]
    order = np.argsort(dst, kind="stable")
    src_s = src[order].astype(np.int32)
    dst_s = dst[order].astype(np.int32)

    tile_g = dst_s // P  # global tile id
    core_e = tile_g // TILES
    t_e = tile_g % TILES

    grp = core_e * TILES + t_e
    cnt = np.bincount(grp, minlength=NCORES * TILES).reshape(NCORES, TILES)
    chunks = np.maximum(1, -(-cnt.max(axis=0) // P)).astype(np.int64)  # [TILES]
    assert chunks.max() <= CHUNK_CAP
    base = np.concatenate([[0], np.cumsum(chunks)])
    TC = int(base[-1])

    starts = np.concatenate([[0], np.cumsum(cnt.ravel())])
    rank = np.arange(E, dtype=np.int64) - starts[grp]
    chunk_e = rank // P
    part_e = rank % P
    col_e = base[t_e] + chunk_e

    srcI = np.zeros((NCORES, P, TC), np.int32)
    dstI = np.zeros((NCORES, P, TC), np.int32)
    dloc = np.full((NCORES, P, TC), 200.0, np.float32)
    srcI[core_e, part_e, col_e] = src_s
    dstI[core_e, part_e, col_e] = dst_s
    dloc[core_e, part_e, col_e] = (dst_s - (core_e * NPC + t_e * P)).astype(
        np.float32
    )

    x_pad = np.zeros((NPAD, IN_DIM), np.float32)
    x_pad[:N_NODES] = x

    W_comb = np.concatenate([W_l, W_r], axis=0)  # [128, 512]
    u_l = np.einsum("ihc,hc->ih", W_l.reshape(IN_DIM, HEADS, OUT_C), att)
    u_r = np.einsum("ihc,hc->ih", W_r.reshape(IN_DIM, HEADS, OUT_C), att)
    u_comb = (NEG * np.concatenate([u_l, u_r], axis=0)).astype(np.float32)  # [128,8]

    att_pads = np.zeros((P, 4, HEADS), np.float32)
    for c in range(4):
        for half in range(2):
            h = 2 * c + half
            att_pads[half * 64 : (half + 1) * 64, c, h] = (1.0 - NEG) * att[h, :]
    att_pads = att_pads.reshape(P, 4 * HEADS)  # [128, 32]

    Wld = np.zeros((P, 4, P), np.float32)  # block-diag W_l pairs
    for c in range(4):
        for half in range(2):
            h = 2 * c + half
            Wld[half * 64 : (half + 1) * 64, c, half * 64 : (half + 1) * 64] = W_l[
                :, h * OUT_C : (h + 1) * OUT_C
            ]
    Wld = Wld.reshape(P, 4 * P)  # [128, 512]

    bias_rep = np.tile(bias[None, :], (P, 1)).astype(np.float32)
    iota_row = np.tile(np.arange(P, dtype=np.float32)[None, :], (P, 1))

    shared = dict(
        x=x_pad, wcomb=W_comb, wld=Wld, ucomb=u_comb, attp=att_pads,
        biasr=bias_rep, iota=iota_row,
    )
    in_maps = []
    for k in range(NCORES):
        m = dict(shared)
        m["srci"] = srcI[k]
        m["dsti"] = dstI[k]
        m["dloc"] = dloc[k]
        in_maps.append(m)
    return in_maps, chunks, base, TC


# ------------------------------------------------------------- kernel builder
def _build_nc(chunks, base, TC):
    from contextlib import ExitStack

    import concourse.bass as bass
    import concourse.tile as tile
    from concourse import bacc, mybir
    from concourse.masks import make_identity

    f32 = mybir.dt.float32
    i32 = mybir.dt.int32
    Alu = mybir.AluOpType
    Act = mybir.ActivationFunctionType

    nc = bacc.Bacc(
        "TRN2", target_bir_lowering=False, debug=False, num_devices=NCORES
    )

    x_d = nc.dram_tensor("x", [NPAD, IN_DIM], f32, kind="ExternalInput").ap()
    wcomb_d = nc.dram_tensor("wcomb", [P, HC], f32, kind="ExternalInput").ap()
    wld_d = nc.dram_tensor("wld", [P, HC], f32, kind="ExternalInput").ap()
    ucomb_d = nc.dram_tensor("ucomb", [P, HEADS], f32, kind="ExternalInput").ap()
    attp_d = nc.dram_tensor("attp", [P, 4 * HEADS], f32, kind="ExternalInput").ap()
    biasr_d = nc.dram_tensor("biasr", [P, HC], f32, kind="ExternalInput").ap()
    iota_d = nc.dram_tensor("iota", [P, P], f32, kind="ExternalInput").ap()
    srci_d = nc.dram_tensor("srci", [P, TC], i32, kind="ExternalInput").ap()
    dsti_d = nc.dram_tensor("dsti", [P, TC], i32, kind="ExternalInput").ap()
    dloc_d = nc.dram_tensor("dloc", [P, TC], f32, kind="ExternalInput").ap()
    out_d = nc.dram_tensor("out", [NPC, HC], f32, kind="ExternalOutput").ap()

    def bc(ap, newap):
        return bass.AP(ap.tensor, ap.offset, newap)

    with tile.TileContext(nc) as tc, ExitStack() as ctx:
        const = ctx.enter_context(tc.tile_pool(name="const", bufs=1))
        wcomb = const.tile([P, HC], f32)
        wld = const.tile([P, HC], f32)
        ucomb = const.tile([P, HEADS], f32)
        attp = const.tile([P, 4 * HEADS], f32)
        biasr = const.tile([P, HC], f32)
        iota = const.tile([P, P], f32)
        srci = const.tile([P, TC], i32)
        dsti = const.tile([P, TC], i32)
        dloc = const.tile([P, TC], f32)
        ident = const.tile([P, P], f32)
        nc.sync.dma_start(wcomb[:], wcomb_d)
        nc.sync.dma_start(wld[:], wld_d)
        nc.sync.dma_start(ucomb[:], ucomb_d)
        nc.sync.dma_start(attp[:], attp_d)
        nc.sync.dma_start(biasr[:], biasr_d)
        nc.sync.dma_start(iota[:], iota_d)
        nc.sync.dma_start(srci[:], srci_d)
        nc.sync.dma_start(dsti[:], dsti_d)
        nc.sync.dma_start(dloc[:], dloc_d)
        make_identity(nc, ident[:])

        CH_MAX = int(max(chunks))
        ps = ctx.enter_context(tc.tile_pool(name="ps", bufs=8, space="PSUM"))
        xsd_p = ctx.enter_context(tc.tile_pool(name="xsd", bufs=2))
        xt_p = ctx.enter_context(tc.tile_pool(name="xt", bufs=3))
        oh_p = ctx.enter_context(tc.tile_pool(name="oh", bufs=3))
        relu_p = ctx.enter_context(tc.tile_pool(name="relu", bufs=3))
        ext_p = ctx.enter_context(tc.tile_pool(name="ext", bufs=3))
        ex_p = ctx.enter_context(tc.tile_pool(name="ex", bufs=3))
        z_p = ctx.enter_context(tc.tile_pool(name="z", bufs=3))
        r_p = ctx.enter_context(tc.tile_pool(name="r", bufs=2))
        nm_p = ctx.enter_context(tc.tile_pool(name="nm", bufs=2))
        at_p = ctx.enter_context(tc.tile_pool(name="at", bufs=2))
        ob_p = ctx.enter_context(tc.tile_pool(name="ob", bufs=2))

        for t in range(TILES):
            CH = int(chunks[t])
            c0 = int(base[t])

            # -- gather x[src] and x[dst] rows into one tile [128, CH, 128].
            # HW indirect DMA supports one index per partition per op, so
            # issue one gather per chunk per side.
            xsd = xsd_p.tile([P, CH_MAX, P], f32, tag="xsd")
            for j in range(CH):
                col = c0 + j
                nc.gpsimd.indirect_dma_start(
                    out=xsd[:, j, 0:IN_DIM],
                    out_offset=None,
                    in_=x_d,
                    in_offset=bass.IndirectOffsetOnAxis(
                        ap=srci[:, col : col + 1], axis=0
                    ),
                )
                nc.gpsimd.indirect_dma_start(
                    out=xsd[:, j, IN_DIM:P],
                    out_offset=None,
                    in_=x_d,
                    in_offset=bass.IndirectOffsetOnAxis(
                        ap=dsti[:, col : col + 1], axis=0
                    ),
                )

            agg_ps = ps.tile([P, HC], f32, tag="ps")
            dn_ps = ps.tile([P, HEADS], f32, tag="ps")

            for j in range(CH):
                col = c0 + j
                # one-hot [128e, 128d]
                oh = oh_p.tile([P, P], f32, tag="oh")
                nc.vector.tensor_scalar(
                    out=oh[:],
                    in0=iota[:],
                    scalar1=dloc[:, col : col + 1],
                    scalar2=None,
                    op0=Alu.is_equal,
                )

                # transpose gathered rows -> xT [128feat, 128e]
                xt_ps = ps.tile([P, P], f32, tag="ps")
                nc.tensor.transpose(xt_ps[:], xsd[:, j, :], ident[:])
                xt = xt_p.tile([P, P], f32, tag="xt")
                nc.vector.tensor_copy(xt[:], xt_ps[:])

                # sT blocks: [128ch_c, 128e] for 4 channel chunks (one PSUM bank)
                st_ps = ps.tile([P, 4, P], f32, tag="ps")
                for c in range(4):
                    nc.tensor.matmul(
                        st_ps[:, c, :],
                        lhsT=wcomb[:, c * P : (c + 1) * P],
                        rhs=xt[:],
                        start=True,
                        stop=True,
                    )
                relu = relu_p.tile([P, 4, P], f32, tag="relu")
                nc.scalar.activation(relu[:], st_ps[:], Act.Relu)

                # scores eT [8, 128e] = 0.2*att.s + 0.8*att.relu(s)
                et_ps = ps.tile([HEADS, P], f32, tag="ps")
                nc.tensor.matmul(
                    et_ps[:], lhsT=ucomb[:], rhs=xt[:], start=True, stop=False
                )
                for c in range(4):
                    nc.tensor.matmul(
                        et_ps[:],
                        lhsT=attp[:, c * HEADS : (c + 1) * HEADS],
                        rhs=relu[:, c, :],
                        start=False,
                        stop=(c == 3),
                    )
                ext = ext_p.tile([HEADS, P], f32, tag="ext")
                nc.scalar.activation(ext[:], et_ps[:], Act.Exp)

                # transpose scores -> ex [128e, 8]
                ex_ps = ps.tile([P, HEADS], f32, tag="ps")
                nc.tensor.transpose(ex_ps[:], ext[:], ident[0:HEADS, 0:HEADS])
                ex = ex_p.tile([P, HEADS], f32, tag="ex")
                nc.vector.tensor_copy(ex[:], ex_ps[:])

                # Z = ex (outer) x_src : [128e, 8, 64]; heads 0-3 on DVE, 4-7 on POOL
                z = z_p.tile([P, HEADS, OUT_C], f32, tag="z")
                xsrc = xsd[:, j, 0:IN_DIM]
                e0 = ex[:, 0:4]
                e4 = ex[:, 4:8]
                nc.vector.scalar_tensor_tensor(
                    out=z[:, 0:4, :],
                    in0=bc(xsrc, [xsrc.ap[0], [0, 4], xsrc.ap[1]]),
                    scalar=0.0,
                    in1=bc(e0, [e0.ap[0], [1, 4], [0, OUT_C]]),
                    op0=Alu.bypass,
                    op1=Alu.mult,
                )
                nc.gpsimd.tensor_tensor(
                    out=z[:, 4:8, :],
                    in0=bc(xsrc, [xsrc.ap[0], [0, 4], xsrc.ap[1]]),
                    in1=bc(e4, [e4.ap[0], [1, 4], [0, OUT_C]]),
                    op=Alu.mult,
                )

                # denom += onehot^T @ ex ; agg += onehot^T @ Z
                nc.tensor.matmul(
                    dn_ps[:],
                    lhsT=oh[:],
                    rhs=ex[:],
                    start=(j == 0),
                    stop=(j == CH - 1),
                )
                nc.tensor.matmul(
                    agg_ps[:],
                    lhsT=oh[:],
                    rhs=z[:].rearrange("p h c -> p (h c)"),
                    start=(j == 0),
                    stop=(j == CH - 1),
                )

            # -- tile epilogue
            r = r_p.tile([P, HEADS], f32, tag="r")
            nc.vector.tensor_scalar_add(r[:], dn_ps[:], 1e-16)
            nc.vector.reciprocal(r[:], r[:])

            normed = nm_p.tile([P, HC], f32, tag="nm")
            agg_ap = agg_ps[:]
            nc.vector.scalar_tensor_tensor(
                out=bc(normed[:], [normed[:].ap[0], [OUT_C, HEADS], [1, OUT_C]]),
                in0=bc(agg_ap, [agg_ap.ap[0], [OUT_C, HEADS], [1, OUT_C]]),
                scalar=0.0,
                in1=bc(r[:], [r[:].ap[0], [1, HEADS], [0, OUT_C]]),
                op0=Alu.bypass,
                op1=Alu.mult,
            )

            at_ps = ps.tile([P, HC], f32, tag="ps")
            for c in range(4):
                nc.tensor.transpose(
                    at_ps[:, c * P : (c + 1) * P],
                    normed[:, c * P : (c + 1) * P],
                    ident[:],
                )
            aggt = at_p.tile([P, HC], f32, tag="at")
            nc.vector.tensor_copy(aggt[:], at_ps[:])

            out_ps = ps.tile([P, HC], f32, tag="ps")
            for c in range(4):
                nc.tensor.matmul(
                    out_ps[:, c * P : (c + 1) * P],
                    lhsT=aggt[:, c * P : (c + 1) * P],
                    rhs=wld[:, c * P : (c + 1) * P],
                    start=True,
                    stop=True,
                )
            outb = ob_p.tile([P, HC], f32, tag="ob")
            nc.vector.tensor_add(outb[:], out_ps[:], biasr[:])
            nc.sync.dma_start(out_d[t * P : (t + 1) * P, :], outb[:])

    nc.compile()
    return nc


# ------------------------------------------------------------------- runner
class _Runner:
    """Builds the PJRT executable once; supports repeated timed execution.

    chain_k > 1 builds an additional jitted function that executes the NEFF
    k times back-to-back inside one dispatch (output i feeds the donated
    output-buffer operands of call i+1, forcing serialization), which
    amortizes the ~100 ms axon RPC floor for timing.
    """

    def __init__(self, nc):
        import jax
        from jax.sharding import Mesh, PartitionSpec
        from jax.experimental.shard_map import shard_map
        from concourse import bass2jax, mybir

        bass2jax.install_neuronx_cc_hook()
        self.jax = jax

        partition_name = (
            nc.partition_id_tensor.name if nc.partition_id_tensor else None
        )
        in_names, out_names, out_avals, zero_outs = [], [], [], []
        for alloc in nc.m.functions[0].allocations:
            if not isinstance(alloc, mybir.MemoryLocationSet):
                continue
            name = alloc.memorylocations[0].name
            if alloc.kind == "ExternalInput":
                if name != partition_name:
                    in_names.append(name)
            elif alloc.kind == "ExternalOutput":
                out_names.append(name)
                shape = tuple(alloc.tensor_shape)
                dtype = mybir.dt.np(alloc.dtype)
                out_avals.append(jax.core.ShapedArray(shape, dtype))
                zero_outs.append(np.zeros(shape, dtype))
        self.in_names = list(in_names)
        self.out_names = out_names
        n_params = len(in_names)
        all_names = in_names + out_names
        if partition_name is not None:
            all_names = all_names + [partition_name]

        def _body(*args):
            operands = list(args)
            if partition_name is not None:
                operands.append(bass2jax.partition_id_tensor())
            outs = bass2jax._bass_exec_p.bind(
                *operands,
                out_avals=tuple(out_avals),
                in_names=tuple(all_names),
                out_names=tuple(out_names),
                lowering_input_output_aliases=(),
                sim_require_finite=False,
                sim_require_nnan=False,
                nc=nc,
            )
            return tuple(outs)

        devices = jax.devices()[:NCORES]
        assert len(devices) == NCORES
        mesh = Mesh(np.asarray(devices), ("core",))
        specs = (PartitionSpec("core"),) * (n_params + len(out_names))
        self.fn = jax.jit(
            shard_map(
                _body,
                mesh=mesh,
                in_specs=specs,
                out_specs=(PartitionSpec("core"),) * len(out_names),
                check_rep=False,
            ),
            keep_unused=True,
        )
        self.zero_outs = zero_outs
        self.mesh = mesh

        n_outs = len(out_names)

        def _body_k(k):
            def f(*args):
                ins = list(args[:n_params])
                zouts = list(args[n_params:])
                for _ in range(k):
                    zouts = list(_body(*ins, *zouts))
                return tuple(zouts)

            return f

        self._mk_chain = lambda k: jax.jit(
            shard_map(
                _body_k(k),
                mesh=mesh,
                in_specs=specs,
                out_specs=(PartitionSpec("core"),) * n_outs,
                check_rep=False,
            ),
            keep_unused=True,
        )
        self._chains = {}

    def time_async(self, args, n=40, trials=4):
        """Marginal per-exec wall time with async-pipelined dispatch."""
        import time

        o = self.fn(*args)
        self.jax.block_until_ready(o)
        res = []
        for _ in range(trials):
            t0 = time.perf_counter()
            outs = None
            for _ in range(n):
                outs = self.fn(*args)
            self.jax.block_until_ready(outs)
            res.append((time.perf_counter() - t0) / n)
        return res

    def prepare(self, in_maps):
        jax = self.jax
        from jax.sharding import NamedSharding, PartitionSpec

        sh = NamedSharding(self.mesh, PartitionSpec("core"))
        args = []
        for name in self.in_names:
            glob = np.concatenate([m[name] for m in in_maps], axis=0)
            args.append(jax.device_put(glob, sh))
        for z in self.zero_outs:
            glob = np.concatenate([z] * NCORES, axis=0)
            args.append(jax.device_put(glob, sh))
        return args

    def run(self, args):
        outs = self.fn(*args)
        self.jax.block_until_ready(outs)
        return [np.asarray(o) for o in outs]

    def time_exec(self, args, iters=10):
        import time

        self.run(args)  # warm
        times = []
        for _ in range(iters):
            t0 = time.perf_counter()
            outs = self.fn(*args)
            self.jax.block_until_ready(outs)
            times.append(time.perf_counter() - t0)
        return times


def _get_compiled(x, edge_index, W_l, W_r, att, bias):
    in_maps, chunks, base, TC = _preprocess(x, edge_index, W_l, W_r, att, bias)
    key = (TC, tuple(int(c) for c in chunks))
    if key not in _COMPILED:
        nc = _build_nc(chunks, base, TC)
        _COMPILED[key] = (nc, _Runner(nc))
    nc, runner = _COMPILED[key]
    return runner, in_maps


def kernel(x, edge_index, W_l, W_r, att, bias):
    runner, in_maps = _get_compiled(x, edge_index, W_l, W_r, att, bias)
    args = runner.prepare(in_maps)
    outs = runner.run(args)
    full = outs[runner.out_names.index("out")]  # [NPAD, 512]
    return full[:N_NODES]


def benchmark(x, edge_index, W_l, W_r, att, bias, n=40, trials=4):
    """Returns (output, list of marginal per-exec wall seconds)."""
    runner, in_maps = _get_compiled(x, edge_index, W_l, W_r, att, bias)
    args = runner.prepare(in_maps)
    outs = runner.run(args)
    full = outs[runner.out_names.index("out")][:N_NODES]
    times = runner.time_async(args, n=n, trials=trials)
    return full, times


# revision 14
# speedup vs baseline: 1.1959x; 1.1959x over previous
"""GATv2Conv forward on 8 Trainium2 NeuronCores (Bass/Tile).

Strategy (dst-sharded, edge-gather, no collectives):
  - Host sorts edges by destination node; core k owns dst nodes
    [k*12544, (k+1)*12544).  Each core processes its own edges fully
    independently (segment max is skipped: scores are small enough that
    exp() cannot overflow, and softmax is shift-invariant).
  - Per dst tile of 128 nodes, edges are packed into chunks of 128
    (padded; pad edges carry dst_local=200 so they match no one-hot
    column and contribute nothing).
  - Per edge chunk:
      gather x[src], x[dst] rows (indirect DMA)  -> [128e, 128]
      PE transpose                               -> xT [128feat, 128e]
      sT = W_comb^T-ish matmuls                  -> [ch, e] blocks (PSUM)
      e_scores = 0.8*att.relu(s) + 0.2*att.s  via PE matmuls into [8, e]
      exp on ACT, PE transpose back              -> ex [128e, 8]
      denom += onehot^T @ ex ; agg += onehot^T @ (ex (x) x_src)
  - Per tile: normalize agg by 1/denom, transpose, multiply by
    block-diagonal W_l, add bias, DMA out.
Output is assembled (concat + crop) on the host.
"""

import sys

sys.path.insert(0, "/opt/trn_rl_repo")

import numpy as np

N_NODES = 100000
N_EDGES = 800000
IN_DIM = 64
HEADS = 8
OUT_C = 64
HC = HEADS * OUT_C  # 512
NEG = 0.2
P = 128
NCORES = 8
TILES = 98  # dst tiles per core
NPC = TILES * P  # 12544 nodes per core
NPAD = NPC * NCORES  # 100352
CHUNK_CAP = 40  # sanity bound on chunks per tile

_COMPILED = {}  # (TC, chunks tuple) -> (nc, runner)


# ----------------------------------------------------------------- host prep
def _preprocess(x, edge_index, W_l, W_r, att, bias):
    x = np.ascontiguousarray(np.asarray(x, dtype=np.float32))
    ei = np.asarray(edge_index)
    src = np.ascontiguousarray(ei[0]).astype(np.int64)
    dst = np.ascontiguousarray(ei[1]).astype(np.int64)
    W_l = np.asarray(W_l, dtype=np.float32)
    W_r = np.asarray(W_r, dtype=np.float32)
    att = np.asarray(att, dtype=np.float32)
    bias = np.asarray(bias, dtype=np.float32)

    E = src.shape[0]
    order = np.argsort(dst, kind="stable")
    src_s = src[order].astype(np.int32)
    dst_s = dst[order].astype(np.int32)

    tile_g = dst_s // P  # global tile id
    core_e = tile_g // TILES
    t_e = tile_g % TILES

    grp = core_e * TILES + t_e
    cnt = np.bincount(grp, minlength=NCORES * TILES).reshape(NCORES, TILES)
    chunks = np.maximum(1, -(-cnt.max(axis=0) // P)).astype(np.int64)  # [TILES]
    assert chunks.max() <= CHUNK_CAP
    base = np.concatenate([[0], np.cumsum(chunks)])
    TC = int(base[-1])

    starts = np.concatenate([[0], np.cumsum(cnt.ravel())])
    rank = np.arange(E, dtype=np.int64) - starts[grp]
    chunk_e = rank // P
    part_e = rank % P
    col_e = base[t_e] + chunk_e

    srcI = np.zeros((NCORES, P, TC), np.int32)
    dstI = np.zeros((NCORES, P, TC), np.int32)
    dloc = np.full((NCORES, P, TC), 200.0, np.float32)
    srcI[core_e, part_e, col_e] = src_s
    dstI[core_e, part_e, col_e] = dst_s
    dloc[core_e, part_e, col_e] = (dst_s - (core_e * NPC + t_e * P)).astype(
        np.float32
    )

    x_pad = np.zeros((NPAD, IN_DIM), np.float32)
    x_pad[:N_NODES] = x

    W_comb = np.concatenate([W_l, W_r], axis=0)  # [128, 512]
    u_l = np.einsum("ihc,hc->ih", W_l.reshape(IN_DIM, HEADS, OUT_C), att)
    u_r = np.einsum("ihc,hc->ih", W_r.reshape(IN_DIM, HEADS, OUT_C), att)
    u_comb = (NEG * np.concatenate([u_l, u_r], axis=0)).astype(np.float32)  # [128,8]

    att_pads = np.zeros((P, 4, HEADS), np.float32)
    for c in range(4):
        for half in range(2):
            h = 2 * c + half
            att_pads[half * 64 : (half + 1) * 64, c, h] = (1.0 - NEG) * att[h, :]
    att_pads = att_pads.reshape(P, 4 * HEADS)  # [128, 32]

    Wld = np.zeros((P, 4, P), np.float32)  # block-diag W_l pairs
    for c in range(4):
        for half in range(2):
            h = 2 * c + half
            Wld[half * 64 : (half + 1) * 64, c, half * 64 : (half + 1) * 64] = W_l[
                :, h * OUT_C : (h + 1) * OUT_C
            ]
    Wld = Wld.reshape(P, 4 * P)  # [128, 512]

    bias_rep = np.tile(bias[None, :], (P, 1)).astype(np.float32)
    iota_row = np.tile(np.arange(P, dtype=np.float32)[None, :], (P, 1))

    shared = dict(
        x=x_pad, wcomb=W_comb, wld=Wld, ucomb=u_comb, attp=att_pads,
        biasr=bias_rep, iota=iota_row,
    )
    in_maps = []
    for k in range(NCORES):
        m = dict(shared)
        m["srci"] = srcI[k]
        m["dsti"] = dstI[k]
        m["dloc"] = dloc[k]
        in_maps.append(m)
    return in_maps, chunks, base, TC


# ------------------------------------------------------------- kernel builder
def _build_nc(chunks, base, TC):
    from contextlib import ExitStack

    import concourse.bass as bass
    import concourse.tile as tile
    from concourse import bacc, mybir
    from concourse.masks import make_identity

    f32 = mybir.dt.float32
    i32 = mybir.dt.int32
    Alu = mybir.AluOpType
    Act = mybir.ActivationFunctionType

    nc = bacc.Bacc(
        "TRN2", target_bir_lowering=False, debug=False, num_devices=NCORES
    )

    x_d = nc.dram_tensor("x", [NPAD, IN_DIM], f32, kind="ExternalInput").ap()
    wcomb_d = nc.dram_tensor("wcomb", [P, HC], f32, kind="ExternalInput").ap()
    wld_d = nc.dram_tensor("wld", [P, HC], f32, kind="ExternalInput").ap()
    ucomb_d = nc.dram_tensor("ucomb", [P, HEADS], f32, kind="ExternalInput").ap()
    attp_d = nc.dram_tensor("attp", [P, 4 * HEADS], f32, kind="ExternalInput").ap()
    biasr_d = nc.dram_tensor("biasr", [P, HC], f32, kind="ExternalInput").ap()
    iota_d = nc.dram_tensor("iota", [P, P], f32, kind="ExternalInput").ap()
    srci_d = nc.dram_tensor("srci", [P, TC], i32, kind="ExternalInput").ap()
    dsti_d = nc.dram_tensor("dsti", [P, TC], i32, kind="ExternalInput").ap()
    dloc_d = nc.dram_tensor("dloc", [P, TC], f32, kind="ExternalInput").ap()
    out_d = nc.dram_tensor("out", [NPC, HC], f32, kind="ExternalOutput").ap()

    def bc(ap, newap):
        return bass.AP(ap.tensor, ap.offset, newap)

    with tile.TileContext(nc) as tc, ExitStack() as ctx:
        const = ctx.enter_context(tc.tile_pool(name="const", bufs=1))
        wcomb = const.tile([P, HC], f32)
        wld = const.tile([P, HC], f32)
        ucomb = const.tile([P, HEADS], f32)
        attp = const.tile([P, 4 * HEADS], f32)
        biasr = const.tile([P, HC], f32)
        iota = const.tile([P, P], f32)
        srci = const.tile([P, TC], i32)
        dsti = const.tile([P, TC], i32)
        dloc = const.tile([P, TC], f32)
        ident = const.tile([P, P], f32)
        nc.sync.dma_start(wcomb[:], wcomb_d)
        nc.sync.dma_start(wld[:], wld_d)
        nc.sync.dma_start(ucomb[:], ucomb_d)
        nc.sync.dma_start(attp[:], attp_d)
        nc.sync.dma_start(biasr[:], biasr_d)
        nc.sync.dma_start(iota[:], iota_d)
        nc.sync.dma_start(srci[:], srci_d)
        nc.sync.dma_start(dsti[:], dsti_d)
        nc.sync.dma_start(dloc[:], dloc_d)
        make_identity(nc, ident[:])

        CH_MAX = int(max(chunks))
        ps = ctx.enter_context(tc.tile_pool(name="ps", bufs=4, space="PSUM"))
        ps_acc = ctx.enter_context(tc.tile_pool(name="psacc", bufs=2, space="PSUM"))
        ps_dn = ctx.enter_context(tc.tile_pool(name="psdn", bufs=2, space="PSUM"))
        xsd_p = ctx.enter_context(tc.tile_pool(name="xsd", bufs=2))
        xt_p = ctx.enter_context(tc.tile_pool(name="xt", bufs=3))
        oh_p = ctx.enter_context(tc.tile_pool(name="oh", bufs=3))
        relu_p = ctx.enter_context(tc.tile_pool(name="relu", bufs=3))
        ext_p = ctx.enter_context(tc.tile_pool(name="ext", bufs=3))
        ex_p = ctx.enter_context(tc.tile_pool(name="ex", bufs=3))
        z_p = ctx.enter_context(tc.tile_pool(name="z", bufs=3))
        r_p = ctx.enter_context(tc.tile_pool(name="r", bufs=2))
        nm_p = ctx.enter_context(tc.tile_pool(name="nm", bufs=2))
        at_p = ctx.enter_context(tc.tile_pool(name="at", bufs=2))
        ob_p = ctx.enter_context(tc.tile_pool(name="ob", bufs=2))

        for t in range(TILES):
            CH = int(chunks[t])
            c0 = int(base[t])

            # -- gather x[src] and x[dst] rows into one tile [128, CH, 128].
            # HW indirect DMA supports one index per partition per op, so
            # issue one gather per chunk per side.
            xsd = xsd_p.tile([P, CH_MAX, P], f32, tag="xsd")
            for j in range(CH):
                col = c0 + j
                nc.gpsimd.indirect_dma_start(
                    out=xsd[:, j, 0:IN_DIM],
                    out_offset=None,
                    in_=x_d,
                    in_offset=bass.IndirectOffsetOnAxis(
                        ap=srci[:, col : col + 1], axis=0
                    ),
                )
                nc.gpsimd.indirect_dma_start(
                    out=xsd[:, j, IN_DIM:P],
                    out_offset=None,
                    in_=x_d,
                    in_offset=bass.IndirectOffsetOnAxis(
                        ap=dsti[:, col : col + 1], axis=0
                    ),
                )

            agg_ps = ps_acc.tile([P, HC], f32, tag="psacc")
            dn_ps = ps_dn.tile([P, HEADS], f32, tag="psdn")

            for j in range(CH):
                col = c0 + j
                # one-hot [128e, 128d]
                oh = oh_p.tile([P, P], f32, tag="oh")
                nc.vector.tensor_scalar(
                    out=oh[:],
                    in0=iota[:],
                    scalar1=dloc[:, col : col + 1],
                    scalar2=None,
                    op0=Alu.is_equal,
                )

                # transpose gathered rows -> xT [128feat, 128e]
                xt_ps = ps.tile([P, P], f32, tag="ps")
                nc.tensor.transpose(xt_ps[:], xsd[:, j, :], ident[:])
                xt = xt_p.tile([P, P], f32, tag="xt")
                nc.vector.tensor_copy(xt[:], xt_ps[:])

                # sT blocks: [128ch_c, 128e] for 4 channel chunks (one PSUM bank)
                st_ps = ps.tile([P, 4, P], f32, tag="ps")
                for c in range(4):
                    nc.tensor.matmul(
                        st_ps[:, c, :],
                        lhsT=wcomb[:, c * P : (c + 1) * P],
                        rhs=xt[:],
                        start=True,
                        stop=True,
                    )
                relu = relu_p.tile([P, 4, P], f32, tag="relu")
                nc.scalar.activation(relu[:], st_ps[:], Act.Relu)

                # scores eT [8, 128e] = 0.2*att.s + 0.8*att.relu(s)
                et_ps = ps.tile([HEADS, P], f32, tag="ps")
                nc.tensor.matmul(
                    et_ps[:], lhsT=ucomb[:], rhs=xt[:], start=True, stop=False
                )
                for c in range(4):
                    nc.tensor.matmul(
                        et_ps[:],
                        lhsT=attp[:, c * HEADS : (c + 1) * HEADS],
                        rhs=relu[:, c, :],
                        start=False,
                        stop=(c == 3),
                    )
                ext = ext_p.tile([HEADS, P], f32, tag="ext")
                nc.scalar.activation(ext[:], et_ps[:], Act.Exp)

                # transpose scores -> ex [128e, 8]
                ex_ps = ps.tile([P, HEADS], f32, tag="ps")
                nc.tensor.transpose(ex_ps[:], ext[:], ident[0:HEADS, 0:HEADS])
                ex = ex_p.tile([P, HEADS], f32, tag="ex")
                nc.vector.tensor_copy(ex[:], ex_ps[:])

                # Z = ex (outer) x_src : [128e, 8, 64]; heads 0-3 on DVE, 4-7 on POOL
                z = z_p.tile([P, HEADS, OUT_C], f32, tag="z")
                xsrc = xsd[:, j, 0:IN_DIM]
                e0 = ex[:]
                nc.vector.scalar_tensor_tensor(
                    out=z[:],
                    in0=bc(xsrc, [xsrc.ap[0], [0, HEADS], xsrc.ap[1]]),
                    scalar=0.0,
                    in1=bc(e0, [e0.ap[0], [1, HEADS], [0, OUT_C]]),
                    op0=Alu.bypass,
                    op1=Alu.mult,
                )

                # denom += onehot^T @ ex ; agg += onehot^T @ Z
                nc.tensor.matmul(
                    dn_ps[:],
                    lhsT=oh[:],
                    rhs=ex[:],
                    start=(j == 0),
                    stop=(j == CH - 1),
                )
                nc.tensor.matmul(
                    agg_ps[:],
                    lhsT=oh[:],
                    rhs=z[:].rearrange("p h c -> p (h c)"),
                    start=(j == 0),
                    stop=(j == CH - 1),
                )

            # -- tile epilogue
            r = r_p.tile([P, HEADS], f32, tag="r")
            nc.vector.tensor_scalar_add(r[:], dn_ps[:], 1e-16)
            nc.vector.reciprocal(r[:], r[:])

            normed = nm_p.tile([P, HC], f32, tag="nm")
            agg_ap = agg_ps[:]
            nc.vector.scalar_tensor_tensor(
                out=bc(normed[:], [normed[:].ap[0], [OUT_C, HEADS], [1, OUT_C]]),
                in0=bc(agg_ap, [agg_ap.ap[0], [OUT_C, HEADS], [1, OUT_C]]),
                scalar=0.0,
                in1=bc(r[:], [r[:].ap[0], [1, HEADS], [0, OUT_C]]),
                op0=Alu.bypass,
                op1=Alu.mult,
            )

            at_ps = ps.tile([P, HC], f32, tag="ps")
            for c in range(4):
                nc.tensor.transpose(
                    at_ps[:, c * P : (c + 1) * P],
                    normed[:, c * P : (c + 1) * P],
                    ident[:],
                )
            aggt = at_p.tile([P, HC], f32, tag="at")
            nc.vector.tensor_copy(aggt[:], at_ps[:])

            out_ps = ps.tile([P, HC], f32, tag="ps")
            for c in range(4):
                nc.tensor.matmul(
                    out_ps[:, c * P : (c + 1) * P],
                    lhsT=aggt[:, c * P : (c + 1) * P],
                    rhs=wld[:, c * P : (c + 1) * P],
                    start=True,
                    stop=True,
                )
            outb = ob_p.tile([P, HC], f32, tag="ob")
            nc.vector.tensor_add(outb[:], out_ps[:], biasr[:])
            nc.sync.dma_start(out_d[t * P : (t + 1) * P, :], outb[:])

    nc.compile()
    return nc


# ------------------------------------------------------------------- runner
class _Runner:
    """Builds the PJRT executable once; supports repeated timed execution.

    chain_k > 1 builds an additional jitted function that executes the NEFF
    k times back-to-back inside one dispatch (output i feeds the donated
    output-buffer operands of call i+1, forcing serialization), which
    amortizes the ~100 ms axon RPC floor for timing.
    """

    def __init__(self, nc):
        import jax
        from jax.sharding import Mesh, PartitionSpec
        from jax.experimental.shard_map import shard_map
        from concourse import bass2jax, mybir

        bass2jax.install_neuronx_cc_hook()
        self.jax = jax

        partition_name = (
            nc.partition_id_tensor.name if nc.partition_id_tensor else None
        )
        in_names, out_names, out_avals, zero_outs = [], [], [], []
        for alloc in nc.m.functions[0].allocations:
            if not isinstance(alloc, mybir.MemoryLocationSet):
                continue
            name = alloc.memorylocations[0].name
            if alloc.kind == "ExternalInput":
                if name != partition_name:
                    in_names.append(name)
            elif alloc.kind == "ExternalOutput":
                out_names.append(name)
                shape = tuple(alloc.tensor_shape)
                dtype = mybir.dt.np(alloc.dtype)
                out_avals.append(jax.core.ShapedArray(shape, dtype))
                zero_outs.append(np.zeros(shape, dtype))
        self.in_names = list(in_names)
        self.out_names = out_names
        n_params = len(in_names)
        all_names = in_names + out_names
        if partition_name is not None:
            all_names = all_names + [partition_name]

        def _body(*args):
            operands = list(args)
            if partition_name is not None:
                operands.append(bass2jax.partition_id_tensor())
            outs = bass2jax._bass_exec_p.bind(
                *operands,
                out_avals=tuple(out_avals),
                in_names=tuple(all_names),
                out_names=tuple(out_names),
                lowering_input_output_aliases=(),
                sim_require_finite=False,
                sim_require_nnan=False,
                nc=nc,
            )
            return tuple(outs)

        devices = jax.devices()[:NCORES]
        assert len(devices) == NCORES
        mesh = Mesh(np.asarray(devices), ("core",))
        specs = (PartitionSpec("core"),) * (n_params + len(out_names))
        self.fn = jax.jit(
            shard_map(
                _body,
                mesh=mesh,
                in_specs=specs,
                out_specs=(PartitionSpec("core"),) * len(out_names),
                check_rep=False,
            ),
            keep_unused=True,
        )
        self.zero_outs = zero_outs
        self.mesh = mesh

        n_outs = len(out_names)

        def _body_k(k):
            def f(*args):
                ins = list(args[:n_params])
                zouts = list(args[n_params:])
                for _ in range(k):
                    zouts = list(_body(*ins, *zouts))
                return tuple(zouts)

            return f

        self._mk_chain = lambda k: jax.jit(
            shard_map(
                _body_k(k),
                mesh=mesh,
                in_specs=specs,
                out_specs=(PartitionSpec("core"),) * n_outs,
                check_rep=False,
            ),
            keep_unused=True,
        )
        self._chains = {}

    def time_async(self, args, n=40, trials=4):
        """Marginal per-exec wall time with async-pipelined dispatch."""
        import time

        o = self.fn(*args)
        self.jax.block_until_ready(o)
        res = []
        for _ in range(trials):
            t0 = time.perf_counter()
            outs = None
            for _ in range(n):
                outs = self.fn(*args)
            self.jax.block_until_ready(outs)
            res.append((time.perf_counter() - t0) / n)
        return res

    def prepare(self, in_maps):
        jax = self.jax
        from jax.sharding import NamedSharding, PartitionSpec

        sh = NamedSharding(self.mesh, PartitionSpec("core"))
        args = []
        for name in self.in_names:
            glob = np.concatenate([m[name] for m in in_maps], axis=0)
            args.append(jax.device_put(glob, sh))
        for z in self.zero_outs:
            glob = np.concatenate([z] * NCORES, axis=0)
            args.append(jax.device_put(glob, sh))
        return args

    def run(self, args):
        outs = self.fn(*args)
        self.jax.block_until_ready(outs)
        return [np.asarray(o) for o in outs]

    def time_exec(self, args, iters=10):
        import time

        self.run(args)  # warm
        times = []
        for _ in range(iters):
            t0 = time.perf_counter()
            outs = self.fn(*args)
            self.jax.block_until_ready(outs)
            times.append(time.perf_counter() - t0)
        return times


def _get_compiled(x, edge_index, W_l, W_r, att, bias):
    in_maps, chunks, base, TC = _preprocess(x, edge_index, W_l, W_r, att, bias)
    key = (TC, tuple(int(c) for c in chunks))
    if key not in _COMPILED:
        nc = _build_nc(chunks, base, TC)
        _COMPILED[key] = (nc, _Runner(nc))
    nc, runner = _COMPILED[key]
    return runner, in_maps


def kernel(x, edge_index, W_l, W_r, att, bias):
    runner, in_maps = _get_compiled(x, edge_index, W_l, W_r, att, bias)
    args = runner.prepare(in_maps)
    outs = runner.run(args)
    full = outs[runner.out_names.index("out")]  # [NPAD, 512]
    return full[:N_NODES]


def benchmark(x, edge_index, W_l, W_r, att, bias, n=40, trials=4):
    """Returns (output, list of marginal per-exec wall seconds)."""
    runner, in_maps = _get_compiled(x, edge_index, W_l, W_r, att, bias)
    args = runner.prepare(in_maps)
    outs = runner.run(args)
    full = outs[runner.out_names.index("out")][:N_NODES]
    times = runner.time_async(args, n=n, trials=trials)
    return full, times


# revision 16
# speedup vs baseline: 1.5475x; 1.2940x over previous
"""GATv2Conv forward on 8 Trainium2 NeuronCores (Bass/Tile).

Strategy (dst-sharded, edge-gather, no collectives):
  - Host sorts edges by destination node; core k owns dst nodes
    [k*12544, (k+1)*12544).  Each core processes its own edges fully
    independently (segment max is skipped: scores are small enough that
    exp() cannot overflow, and softmax is shift-invariant).
  - Per dst tile of 128 nodes, edges are packed into chunks of 128
    (padded; pad edges carry dst_local=200 so they match no one-hot
    column and contribute nothing).
  - Per edge chunk:
      gather x[src], x[dst] rows (indirect DMA)  -> [128e, 128]
      PE transpose                               -> xT [128feat, 128e]
      sT = W_comb^T-ish matmuls                  -> [ch, e] blocks (PSUM)
      e_scores = 0.8*att.relu(s) + 0.2*att.s  via PE matmuls into [8, e]
      exp on ACT, PE transpose back              -> ex [128e, 8]
      denom += onehot^T @ ex ; agg += onehot^T @ (ex (x) x_src)
  - Per tile: normalize agg by 1/denom, transpose, multiply by
    block-diagonal W_l, add bias, DMA out.
Output is assembled (concat + crop) on the host.
"""

import sys

sys.path.insert(0, "/opt/trn_rl_repo")

import numpy as np

N_NODES = 100000
N_EDGES = 800000
IN_DIM = 64
HEADS = 8
OUT_C = 64
HC = HEADS * OUT_C  # 512
NEG = 0.2
P = 128
NCORES = 8
TILES = 98  # dst tiles per core
NPC = TILES * P  # 12544 nodes per core
NPAD = NPC * NCORES  # 100352
CHUNK_CAP = 40  # sanity bound on chunks per tile

_COMPILED = {}  # (TC, chunks tuple) -> (nc, runner)


# ----------------------------------------------------------------- host prep
def _preprocess(x, edge_index, W_l, W_r, att, bias):
    x = np.ascontiguousarray(np.asarray(x, dtype=np.float32))
    ei = np.asarray(edge_index)
    src = np.ascontiguousarray(ei[0]).astype(np.int64)
    dst = np.ascontiguousarray(ei[1]).astype(np.int64)
    W_l = np.asarray(W_l, dtype=np.float32)
    W_r = np.asarray(W_r, dtype=np.float32)
    att = np.asarray(att, dtype=np.float32)
    bias = np.asarray(bias, dtype=np.float32)

    E = src.shape[0]
    order = np.argsort(dst, kind="stable")
    src_s = src[order].astype(np.int32)
    dst_s = dst[order].astype(np.int32)

    tile_g = dst_s // P  # global tile id
    core_e = tile_g // TILES
    t_e = tile_g % TILES

    grp = core_e * TILES + t_e
    cnt = np.bincount(grp, minlength=NCORES * TILES).reshape(NCORES, TILES)
    chunks = np.maximum(1, -(-cnt.max(axis=0) // P)).astype(np.int64)  # [TILES]
    assert chunks.max() <= CHUNK_CAP
    base = np.concatenate([[0], np.cumsum(chunks)])
    TC = int(base[-1])

    starts = np.concatenate([[0], np.cumsum(cnt.ravel())])
    rank = np.arange(E, dtype=np.int64) - starts[grp]
    chunk_e = rank // P
    part_e = rank % P
    col_e = base[t_e] + chunk_e

    srcI = np.zeros((NCORES, P, TC), np.int32)
    dstI = np.zeros((NCORES, P, TC), np.int32)
    dloc = np.full((NCORES, P, TC), 200.0, np.float32)
    srcI[core_e, part_e, col_e] = src_s
    dstI[core_e, part_e, col_e] = dst_s
    dloc[core_e, part_e, col_e] = (dst_s - (core_e * NPC + t_e * P)).astype(
        np.float32
    )

    x_pad = np.zeros((NPAD, IN_DIM), np.float32)
    x_pad[:N_NODES] = x

    W_comb = np.concatenate([W_l, W_r], axis=0)  # [128, 512]
    u_l = np.einsum("ihc,hc->ih", W_l.reshape(IN_DIM, HEADS, OUT_C), att)
    u_r = np.einsum("ihc,hc->ih", W_r.reshape(IN_DIM, HEADS, OUT_C), att)
    u_comb = (NEG * np.concatenate([u_l, u_r], axis=0)).astype(np.float32)  # [128,8]

    att_pads = np.zeros((P, 4, HEADS), np.float32)
    for c in range(4):
        for half in range(2):
            h = 2 * c + half
            att_pads[half * 64 : (half + 1) * 64, c, h] = (1.0 - NEG) * att[h, :]
    att_pads = att_pads.reshape(P, 4 * HEADS)  # [128, 32]

    Wld = np.zeros((P, 4, P), np.float32)  # block-diag W_l pairs
    for c in range(4):
        for half in range(2):
            h = 2 * c + half
            Wld[half * 64 : (half + 1) * 64, c, half * 64 : (half + 1) * 64] = W_l[
                :, h * OUT_C : (h + 1) * OUT_C
            ]
    Wld = Wld.reshape(P, 4 * P)  # [128, 512]

    bias_rep = np.tile(bias[None, :], (P, 1)).astype(np.float32)
    iota_row = np.tile(np.arange(P, dtype=np.float32)[None, :], (P, 1))

    shared = dict(
        x=x_pad, wcomb=W_comb, wld=Wld, ucomb=u_comb, attp=att_pads,
        biasr=bias_rep, iota=iota_row,
    )
    in_maps = []
    for k in range(NCORES):
        m = dict(shared)
        m["srci"] = srcI[k]
        m["dsti"] = dstI[k]
        m["dloc"] = dloc[k]
        in_maps.append(m)
    return in_maps, chunks, base, TC


# ------------------------------------------------------------- kernel builder
def _build_nc(chunks, base, TC):
    import os
    from contextlib import ExitStack

    variant = os.environ.get("KERNEL_VARIANT", "full")

    import concourse.bass as bass
    import concourse.tile as tile
    from concourse import bacc, mybir
    from concourse.masks import make_identity

    f32 = mybir.dt.float32
    i32 = mybir.dt.int32
    Alu = mybir.AluOpType
    Act = mybir.ActivationFunctionType

    nc = bacc.Bacc(
        "TRN2", target_bir_lowering=False, debug=False, num_devices=NCORES
    )

    x_d = nc.dram_tensor("x", [NPAD, IN_DIM], f32, kind="ExternalInput").ap()
    wcomb_d = nc.dram_tensor("wcomb", [P, HC], f32, kind="ExternalInput").ap()
    wld_d = nc.dram_tensor("wld", [P, HC], f32, kind="ExternalInput").ap()
    ucomb_d = nc.dram_tensor("ucomb", [P, HEADS], f32, kind="ExternalInput").ap()
    attp_d = nc.dram_tensor("attp", [P, 4 * HEADS], f32, kind="ExternalInput").ap()
    biasr_d = nc.dram_tensor("biasr", [P, HC], f32, kind="ExternalInput").ap()
    iota_d = nc.dram_tensor("iota", [P, P], f32, kind="ExternalInput").ap()
    srci_d = nc.dram_tensor("srci", [P, TC], i32, kind="ExternalInput").ap()
    dsti_d = nc.dram_tensor("dsti", [P, TC], i32, kind="ExternalInput").ap()
    dloc_d = nc.dram_tensor("dloc", [P, TC], f32, kind="ExternalInput").ap()
    out_d = nc.dram_tensor("out", [NPC, HC], f32, kind="ExternalOutput").ap()

    def bc(ap, newap):
        return bass.AP(ap.tensor, ap.offset, newap)

    with tile.TileContext(nc) as tc, ExitStack() as ctx:
        const = ctx.enter_context(tc.tile_pool(name="const", bufs=1))
        wcomb = const.tile([P, HC], f32)
        wld = const.tile([P, HC], f32)
        ucomb = const.tile([P, HEADS], f32)
        attp = const.tile([P, 4 * HEADS], f32)
        biasr = const.tile([P, HC], f32)
        iota = const.tile([P, P], f32)
        srci = const.tile([P, TC], i32)
        dsti = const.tile([P, TC], i32)
        dloc = const.tile([P, TC], f32)
        ident = const.tile([P, P], f32)
        nc.sync.dma_start(wcomb[:], wcomb_d)
        nc.sync.dma_start(wld[:], wld_d)
        nc.sync.dma_start(ucomb[:], ucomb_d)
        nc.sync.dma_start(attp[:], attp_d)
        nc.sync.dma_start(biasr[:], biasr_d)
        nc.sync.dma_start(iota[:], iota_d)
        nc.sync.dma_start(srci[:], srci_d)
        nc.sync.dma_start(dsti[:], dsti_d)
        nc.sync.dma_start(dloc[:], dloc_d)
        make_identity(nc, ident[:])

        CH_MAX = int(max(chunks))
        ps_work = ctx.enter_context(tc.tile_pool(name="pswork", bufs=3, space="PSUM"))
        ps_st = ctx.enter_context(tc.tile_pool(name="psst", bufs=3, space="PSUM"))
        ps_acc = ctx.enter_context(tc.tile_pool(name="psacc", bufs=1, space="PSUM"))
        ps_dn = ctx.enter_context(tc.tile_pool(name="psdn", bufs=1, space="PSUM"))
        xsd_p = ctx.enter_context(tc.tile_pool(name="xsd", bufs=2))
        xt_p = ctx.enter_context(tc.tile_pool(name="xt", bufs=3))
        oh_p = ctx.enter_context(tc.tile_pool(name="oh", bufs=3))
        relu_p = ctx.enter_context(tc.tile_pool(name="relu", bufs=3))
        ext_p = ctx.enter_context(tc.tile_pool(name="ext", bufs=3))
        ex_p = ctx.enter_context(tc.tile_pool(name="ex", bufs=3))
        z_p = ctx.enter_context(tc.tile_pool(name="z", bufs=3))
        r_p = ctx.enter_context(tc.tile_pool(name="r", bufs=2))
        nm_p = ctx.enter_context(tc.tile_pool(name="nm", bufs=2))
        at_p = ctx.enter_context(tc.tile_pool(name="at", bufs=2))
        ob_p = ctx.enter_context(tc.tile_pool(name="ob", bufs=2))

        for t in range(TILES):
            CH = int(chunks[t])
            c0 = int(base[t])

            # -- gather x[src] and x[dst] rows into one tile [128, CH, 128].
            # HW indirect DMA supports one index per partition per op, so
            # issue one gather per chunk per side.
            xsd = xsd_p.tile([P, CH_MAX, P], f32, tag="xsd")
            if variant == "nogather":
                xr = x_d[0:P, :]
                nc.sync.dma_start(
                    xsd[:, :CH, :],
                    bc(xr, [xr.ap[0], [0, CH * 2], [1, IN_DIM]]),
                )
            else:
                for j in range(CH):
                    col = c0 + j
                    nc.gpsimd.indirect_dma_start(
                        out=xsd[:, j, 0:IN_DIM],
                        out_offset=None,
                        in_=x_d,
                        in_offset=bass.IndirectOffsetOnAxis(
                            ap=srci[:, col : col + 1], axis=0
                        ),
                    )
                    nc.gpsimd.indirect_dma_start(
                        out=xsd[:, j, IN_DIM:P],
                        out_offset=None,
                        in_=x_d,
                        in_offset=bass.IndirectOffsetOnAxis(
                            ap=dsti[:, col : col + 1], axis=0
                        ),
                    )
            if variant == "gatheronly":
                outb = ob_p.tile([P, HC], f32, tag="ob")
                nc.vector.tensor_copy(outb[:], biasr[:])
                nc.sync.dma_start(out_d[t * P : (t + 1) * P, :], outb[:])
                continue

            agg_ps = ps_acc.tile([P, HC], f32, tag="psacc")
            dn_ps = ps_dn.tile([P, HEADS], f32, tag="psdn")

            for j in range(CH):
                col = c0 + j
                # one-hot [128e, 128d]
                oh = oh_p.tile([P, P], f32, tag="oh")
                nc.vector.tensor_scalar(
                    out=oh[:],
                    in0=iota[:],
                    scalar1=dloc[:, col : col + 1],
                    scalar2=None,
                    op0=Alu.is_equal,
                )

                # one work bank holds xt [*,0:128], eT [0:8,128:256], ex [*,256:264]
                work = ps_work.tile([P, 4, P], f32, tag="pswork")
                xt_ps = work[:, 0, :]
                et_ps = work[0:HEADS, 1, :]
                ex_ps = work[:, 2, 0:HEADS]
                # transpose gathered rows -> xT [128feat, 128e]
                nc.tensor.transpose(xt_ps, xsd[:, j, :], ident[:])
                xt = xt_p.tile([P, P], f32, tag="xt")
                nc.vector.tensor_copy(xt[:], xt_ps)

                # sT blocks: [128ch_c, 128e] for 4 channel chunks (one PSUM bank)
                st_ps = ps_st.tile([P, 4, P], f32, tag="psst")
                for c in range(4):
                    nc.tensor.matmul(
                        st_ps[:, c, :],
                        lhsT=wcomb[:, c * P : (c + 1) * P],
                        rhs=xt[:],
                        start=True,
                        stop=True,
                    )
                relu = relu_p.tile([P, 4, P], f32, tag="relu")
                nc.scalar.activation(relu[:], st_ps[:], Act.Relu)

                # scores eT [8, 128e] = 0.2*att.s + 0.8*att.relu(s)
                nc.tensor.matmul(
                    et_ps, lhsT=ucomb[:], rhs=xt[:], start=True, stop=False
                )
                for c in range(4):
                    nc.tensor.matmul(
                        et_ps,
                        lhsT=attp[:, c * HEADS : (c + 1) * HEADS],
                        rhs=relu[:, c, :],
                        start=False,
                        stop=(c == 3),
                    )
                ext = ext_p.tile([HEADS, P], f32, tag="ext")
                nc.scalar.activation(ext[:], et_ps, Act.Exp)

                # transpose scores -> ex [128e, 8]
                nc.tensor.transpose(ex_ps, ext[:], ident[0:HEADS, 0:HEADS])
                ex = ex_p.tile([P, HEADS], f32, tag="ex")
                nc.vector.tensor_copy(ex[:], ex_ps)

                # Z = ex (outer) x_src : [128e, 8, 64]; heads 0-3 on DVE, 4-7 on POOL
                z = z_p.tile([P, HEADS, OUT_C], f32, tag="z")
                xsrc = xsd[:, j, 0:IN_DIM]
                e0 = ex[:]
                nc.vector.scalar_tensor_tensor(
                    out=z[:],
                    in0=bc(xsrc, [xsrc.ap[0], [0, HEADS], xsrc.ap[1]]),
                    scalar=0.0,
                    in1=bc(e0, [e0.ap[0], [1, HEADS], [0, OUT_C]]),
                    op0=Alu.bypass,
                    op1=Alu.mult,
                )

                # denom += onehot^T @ ex ; agg += onehot^T @ Z
                nc.tensor.matmul(
                    dn_ps[:],
                    lhsT=oh[:],
                    rhs=ex[:],
                    start=(j == 0),
                    stop=(j == CH - 1),
                )
                nc.tensor.matmul(
                    agg_ps[:],
                    lhsT=oh[:],
                    rhs=z[:].rearrange("p h c -> p (h c)"),
                    start=(j == 0),
                    stop=(j == CH - 1),
                )

            # -- tile epilogue
            r = r_p.tile([P, HEADS], f32, tag="r")
            nc.vector.tensor_scalar_add(r[:], dn_ps[:], 1e-16)
            nc.vector.reciprocal(r[:], r[:])

            normed = nm_p.tile([P, HC], f32, tag="nm")
            agg_ap = agg_ps[:]
            nc.vector.scalar_tensor_tensor(
                out=bc(normed[:], [normed[:].ap[0], [OUT_C, HEADS], [1, OUT_C]]),
                in0=bc(agg_ap, [agg_ap.ap[0], [OUT_C, HEADS], [1, OUT_C]]),
                scalar=0.0,
                in1=bc(r[:], [r[:].ap[0], [1, HEADS], [0, OUT_C]]),
                op0=Alu.bypass,
                op1=Alu.mult,
            )

            at_ps = ps_st.tile([P, 4, P], f32, tag="psst")
            for c in range(4):
                nc.tensor.transpose(
                    at_ps[:, c, :],
                    normed[:, c * P : (c + 1) * P],
                    ident[:],
                )
            aggt = at_p.tile([P, HC], f32, tag="at")
            nc.vector.tensor_copy(aggt[:], at_ps[:].rearrange("p a b -> p (a b)"))

            out_ps = ps_st.tile([P, 4, P], f32, tag="psst")
            for c in range(4):
                nc.tensor.matmul(
                    out_ps[:, c, :],
                    lhsT=aggt[:, c * P : (c + 1) * P],
                    rhs=wld[:, c * P : (c + 1) * P],
                    start=True,
                    stop=True,
                )
            outb = ob_p.tile([P, HC], f32, tag="ob")
            nc.vector.tensor_add(
                outb[:], out_ps[:].rearrange("p a b -> p (a b)"), biasr[:]
            )
            nc.sync.dma_start(out_d[t * P : (t + 1) * P, :], outb[:])

    nc.compile()
    return nc


# ------------------------------------------------------------------- runner
class _Runner:
    """Builds the PJRT executable once; supports repeated timed execution.

    chain_k > 1 builds an additional jitted function that executes the NEFF
    k times back-to-back inside one dispatch (output i feeds the donated
    output-buffer operands of call i+1, forcing serialization), which
    amortizes the ~100 ms axon RPC floor for timing.
    """

    def __init__(self, nc):
        import jax
        from jax.sharding import Mesh, PartitionSpec
        from jax.experimental.shard_map import shard_map
        from concourse import bass2jax, mybir

        bass2jax.install_neuronx_cc_hook()
        self.jax = jax

        partition_name = (
            nc.partition_id_tensor.name if nc.partition_id_tensor else None
        )
        in_names, out_names, out_avals, zero_outs = [], [], [], []
        for alloc in nc.m.functions[0].allocations:
            if not isinstance(alloc, mybir.MemoryLocationSet):
                continue
            name = alloc.memorylocations[0].name
            if alloc.kind == "ExternalInput":
                if name != partition_name:
                    in_names.append(name)
            elif alloc.kind == "ExternalOutput":
                out_names.append(name)
                shape = tuple(alloc.tensor_shape)
                dtype = mybir.dt.np(alloc.dtype)
                out_avals.append(jax.core.ShapedArray(shape, dtype))
                zero_outs.append(np.zeros(shape, dtype))
        self.in_names = list(in_names)
        self.out_names = out_names
        n_params = len(in_names)
        all_names = in_names + out_names
        if partition_name is not None:
            all_names = all_names + [partition_name]

        def _body(*args):
            operands = list(args)
            if partition_name is not None:
                operands.append(bass2jax.partition_id_tensor())
            outs = bass2jax._bass_exec_p.bind(
                *operands,
                out_avals=tuple(out_avals),
                in_names=tuple(all_names),
                out_names=tuple(out_names),
                lowering_input_output_aliases=(),
                sim_require_finite=False,
                sim_require_nnan=False,
                nc=nc,
            )
            return tuple(outs)

        devices = jax.devices()[:NCORES]
        assert len(devices) == NCORES
        mesh = Mesh(np.asarray(devices), ("core",))
        specs = (PartitionSpec("core"),) * (n_params + len(out_names))
        self.fn = jax.jit(
            shard_map(
                _body,
                mesh=mesh,
                in_specs=specs,
                out_specs=(PartitionSpec("core"),) * len(out_names),
                check_rep=False,
            ),
            keep_unused=True,
        )
        self.zero_outs = zero_outs
        self.mesh = mesh

        n_outs = len(out_names)

        def _body_k(k):
            def f(*args):
                ins = list(args[:n_params])
                zouts = list(args[n_params:])
                for _ in range(k):
                    zouts = list(_body(*ins, *zouts))
                return tuple(zouts)

            return f

        self._mk_chain = lambda k: jax.jit(
            shard_map(
                _body_k(k),
                mesh=mesh,
                in_specs=specs,
                out_specs=(PartitionSpec("core"),) * n_outs,
                check_rep=False,
            ),
            keep_unused=True,
        )
        self._chains = {}

    def time_async(self, args, n=40, trials=4):
        """Marginal per-exec wall time with async-pipelined dispatch."""
        import time

        o = self.fn(*args)
        self.jax.block_until_ready(o)
        res = []
        for _ in range(trials):
            t0 = time.perf_counter()
            outs = None
            for _ in range(n):
                outs = self.fn(*args)
            self.jax.block_until_ready(outs)
            res.append((time.perf_counter() - t0) / n)
        return res

    def prepare(self, in_maps):
        jax = self.jax
        from jax.sharding import NamedSharding, PartitionSpec

        sh = NamedSharding(self.mesh, PartitionSpec("core"))
        args = []
        for name in self.in_names:
            glob = np.concatenate([m[name] for m in in_maps], axis=0)
            args.append(jax.device_put(glob, sh))
        for z in self.zero_outs:
            glob = np.concatenate([z] * NCORES, axis=0)
            args.append(jax.device_put(glob, sh))
        return args

    def run(self, args):
        outs = self.fn(*args)
        self.jax.block_until_ready(outs)
        return [np.asarray(o) for o in outs]

    def time_exec(self, args, iters=10):
        import time

        self.run(args)  # warm
        times = []
        for _ in range(iters):
            t0 = time.perf_counter()
            outs = self.fn(*args)
            self.jax.block_until_ready(outs)
            times.append(time.perf_counter() - t0)
        return times


def _get_compiled(x, edge_index, W_l, W_r, att, bias):
    in_maps, chunks, base, TC = _preprocess(x, edge_index, W_l, W_r, att, bias)
    key = (TC, tuple(int(c) for c in chunks))
    if key not in _COMPILED:
        nc = _build_nc(chunks, base, TC)
        _COMPILED[key] = (nc, _Runner(nc))
    nc, runner = _COMPILED[key]
    return runner, in_maps


def kernel(x, edge_index, W_l, W_r, att, bias):
    runner, in_maps = _get_compiled(x, edge_index, W_l, W_r, att, bias)
    args = runner.prepare(in_maps)
    outs = runner.run(args)
    full = outs[runner.out_names.index("out")]  # [NPAD, 512]
    return full[:N_NODES]


def benchmark(x, edge_index, W_l, W_r, att, bias, n=40, trials=4):
    """Returns (output, list of marginal per-exec wall seconds)."""
    runner, in_maps = _get_compiled(x, edge_index, W_l, W_r, att, bias)
    args = runner.prepare(in_maps)
    outs = runner.run(args)
    full = outs[runner.out_names.index("out")][:N_NODES]
    times = runner.time_async(args, n=n, trials=trials)
    return full, times
